# revision 53
# baseline (speedup 1.0000x reference)
"""Trainium2 Bass kernel for a heterogeneous GraphConv layer (3 relations).

out = concat([leaky(GC(inst_feat, W_inst, in_*)),     # -> node   (10000)
              leaky(GC(node_feat, W_node, ni_*)),     # -> inst   (100000)
              leaky(GC(svc_feat,  W_svc,  sc_*))])    # -> svc    (20000)

GC(f, W, src, dst) = rsqrt(deg_d) * segsum_dst((rsqrt(deg_s)*f)[src]) @ W + b
(aggregation commutes with the dense @W, so we gather *raw scaled features*
and apply W once per destination tile group).

Strategy: destination-sharded across 8 NeuronCores.  The per-core source
tables are PERMUTED so that rows co-used by the same dst tile sit adjacently;
each dma_gather descriptor then uses an overlapping 512B window (elem 256
fp16 elems, step 128) that fetches TWO consecutive rows — one descriptor
serves up to two edges (lanes A/B).  Descriptor cost on TRN2 is identical
for 256B and 512B payloads, so pairing halves gather DMA time.  Gathers are
issued in small (8-block) chunks from a per-relation plan so transfers,
SWDGE descriptor generation and downstream compute pipeline finely.

Edges (sorted by dst) are packed densely into 128-slot blocks with per-tile
slot quotas (max over cores) so the block->tile map is identical on every
core.  Aggregation runs per GROUP of TP=2 dst tiles (256 PSUM columns):
per (block, lane, group) one DVE tensor_scalar builds a value-weighted
one-hot S[slot, d] = rs_dst * (dl == iota+off) (4x_2p DVE mode; the rsqrt
deg_d scale rides the one-hot so the epilogue needs no rank-1 bias matmul),
and PE accumulates agg[f, d] += G_lane.T @ S in PSUM.  Per group: one
matmul po[h, d] = W.T @ agg, one ScalarE Lrelu(po + b[h]) (bias per
partition in the [h, d] orientation), fp16 output DMA in the transposed
[h, d] layout (the host de-transposes and converts).
"""

import os as _os
from collections import defaultdict

import numpy as np

SVC_N, INST_N, NODE_N, HID = 20000, 100000, 10000, 128
NCORES = 8
BLK = 128           # slots per block (= PE contraction dim)
LANES = 2           # table rows per gather window (512B / 256B fp16 rows)
TP = 2              # dst tiles per aggregation group (256 PSUM columns)
CHUNK = int(_os.environ.get("GNN_CHUNK", "24"))   # blocks per gather instr
OUT_GRP = int(_os.environ.get("GNN_OUT_GRP", "16"))  # dst tiles per out DMA
ACT_MODE = "lrelu"

_cache = {}


def _cdiv(a, b):
    return (a + b - 1) // b


def _rup(a, b):
    return _cdiv(a, b) * b


def _sequence_sources(es, tile):
    """Order this core's used sources so same-tileset sources are adjacent."""
    n = len(es)
    starts = np.flatnonzero(np.r_[True, es[1:] != es[:-1]])
    ends = np.r_[starts[1:], n]
    keys = [tuple(tile[a:b]) for a, b in zip(starts, ends)]
    order = sorted(range(len(starts)), key=lambda i: keys[i])
    return order, starts, ends


def _prep_relation(src, dst, n_src, n_dst, feat_s, rs_d, compact):
    """Host-side sharding/packing for one relation."""
    src = np.asarray(src, np.int64)
    dst = np.asarray(dst, np.int64)

    D = _rup(_cdiv(n_dst, NCORES), 128)  # dst rows per core (padded)
    ntiles = D // 128
    assert ntiles % TP == 0

    cores = []
    for c in range(NCORES):
        lo = c * D
        m = (dst >= lo) & (dst < lo + D)
        es, ed = src[m], dst[m] - lo
        tl = ed >> 7
        order = np.lexsort((tl, es))
        es, ed, tl = es[order], ed[order], tl[order]

        uorder, starts, ends = _sequence_sources(es, tl)
        srcs_u = es[starts]
        nsrc_u = len(srcs_u)

        pos_of_u = np.empty(nsrc_u, np.int64)
        pos_of_u[uorder] = np.arange(nsrc_u)

        if compact:
            table = feat_s[srcs_u[uorder]]
            n_units = nsrc_u
        else:
            used_mask = np.zeros(n_src, bool)
            used_mask[srcs_u] = True
            perm = np.concatenate([srcs_u[uorder],
                                   np.flatnonzero(~used_mask)])
            table = feat_s[perm]
            n_units = n_src

        # slots per tile via the path-greedy pairing over table positions
        slot_k = [[] for _ in range(ntiles)]
        slot_dA = [[] for _ in range(ntiles)]
        slot_dB = [[] for _ in range(ntiles)]
        per_tile = defaultdict(list)  # tile -> list of (pos, [dst_locals])
        for ui in range(nsrc_u):
            a, b = starts[ui], ends[ui]
            p = pos_of_u[ui]
            t0 = a
            while t0 < b:
                t1 = t0
                while t1 < b and tl[t1] == tl[t0]:
                    t1 += 1
                per_tile[tl[t0]].append((p, ed[t0:t1]))
                t0 = t1
        for t, lst in per_tile.items():
            lst.sort(key=lambda x: x[0])
            sk, sa, sb = slot_k[t], slot_dA[t], slot_dB[t]
            prev_pos = -10
            prev_ds = []
            for p, ds in lst:
                ds = list(ds)
                if p == prev_pos + 1 and prev_ds:
                    npair = min(len(prev_ds), len(ds))
                    for i in range(npair):
                        sk.append(prev_pos)
                        sa.append(prev_ds[i])
                        sb.append(ds[i])
                    for d in prev_ds[npair:]:
                        sk.append(prev_pos)
                        sa.append(d)
                        sb.append(-1)
                    ds = ds[npair:]
                else:
                    for d in prev_ds:
                        sk.append(prev_pos)
                        sa.append(d)
                        sb.append(-1)
                prev_pos, prev_ds = p, ds
            for d in prev_ds:
                sk.append(prev_pos)
                sa.append(d)
                sb.append(-1)
            # paired slots first so lane-B tails can be skipped
            osort = sorted(range(len(sk)), key=lambda i: sb[i] < 0)
            slot_k[t] = [sk[i] for i in osort]
            slot_dA[t] = [sa[i] for i in osort]
            slot_dB[t] = [sb[i] for i in osort]

        cores.append(dict(slot_k=slot_k, slot_dA=slot_dA, slot_dB=slot_dB,
                          table=table, n_units=n_units))

    # shared per-tile quotas and block map
    quota = np.zeros(ntiles, np.int64)
    for t in range(ntiles):
        quota[t] = max(max(len(cores[c]["slot_k"][t]) for c in range(NCORES)), 1)
    cum = np.concatenate([[0], np.cumsum(quota)])
    nslot = int(cum[-1])
    nslot_pad = _rup(nslot, BLK)
    nblk = nslot_pad // BLK
    bstart = (cum[:-1] // BLK).astype(np.int64)
    bend = np.minimum(-(-cum[1:] // BLK), nblk).astype(np.int64)
    bend = np.maximum(bend, bstart + 1)
    # T0(b): first tile covering block b; span(b): tiles covered
    T0 = np.zeros(nblk, np.int64)
    cur = 0
    for b in range(nblk):
        while bend[cur] <= b:
            cur += 1
        T0[b] = cur
    span = np.ones(nblk, np.int64)
    for t in range(ntiles):
        for b in range(int(bstart[t]), int(bend[t])):
            span[b] = max(span[b], t - T0[b] + 1)

    # per-core dst rsqrt-degree values (0 beyond n_dst)
    rs_core = []
    for c in range(NCORES):
        lo = c * D
        v = np.zeros(D, np.float32)
        n = max(0, min(D, n_dst - lo))
        if n > 0:
            v[:n] = rs_d[lo:lo + n]
        rs_core.append(v)

    ngrp = ntiles // TP
    activeA = np.zeros((ntiles, nblk), bool)
    activeB = np.zeros((ntiles, nblk), bool)
    for c in range(NCORES):
        d = cores[c]
        kidx = np.zeros(nslot_pad, np.int64)
        dA = np.full(nslot_pad, -1.0, np.float32)
        dB = np.full(nslot_pad, -1.0, np.float32)
        rA = np.zeros(nslot_pad, np.float32)
        rB = np.zeros(nslot_pad, np.float32)
        rsv = rs_core[c]
        for t in range(ntiles):
            off = int(cum[t])
            sk, sa, sb = d["slot_k"][t], d["slot_dA"][t], d["slot_dB"][t]
            for i in range(len(sk)):
                b = (off + i) // BLK
                shift = 128 * int(T0[b])
                kidx[off + i] = sk[i]
                dA[off + i] = sa[i] - shift
                rA[off + i] = rsv[sa[i]]
                activeA[t, b] = True
                if sb[i] >= 0:
                    dB[off + i] = sb[i] - shift
                    rB[off + i] = rsv[sb[i]]
                    activeB[t, b] = True
        # tail pads keep idx 0 (cost model charges num_idxs regardless; a
        # real gather keeps the SBUF block initialized -- NaN x 0 hazard)
        d["kidx"], d["dA"], d["dB"], d["rA"], d["rB"] = kidx, dA, dB, rA, rB
        del d["slot_k"], d["slot_dA"], d["slot_dB"]

    # force one active matmul per tile so every agg gets a start+stop
    for t in range(ntiles):
        if not activeA[t, bstart[t]:bend[t]].any() and \
           not activeB[t, bstart[t]:bend[t]].any():
            activeA[t, bstart[t]] = True

    return dict(cores=cores, ntiles=ntiles, ngrp=ngrp, D=D, n_dst=n_dst,
                nslot=nslot, nslot_pad=nslot_pad, nblk=nblk,
                bstart=bstart, bend=bend, T0=T0, span=span,
                activeA=activeA, activeB=activeB)


def _build_host_data(inputs):
    def prescale(feat, src, n_src):
        deg = np.maximum(np.bincount(np.asarray(src, np.int64),
                                     minlength=n_src), 1.0)
        return (np.asarray(feat, np.float32)
                / np.sqrt(deg)[:, None]).astype(np.float32)

    def rs_of(dstv, n_dst):
        deg = np.maximum(np.bincount(np.asarray(dstv, np.int64),
                                     minlength=n_dst), 1.0)
        return (1.0 / np.sqrt(deg)).astype(np.float32)

    feat0 = prescale(inputs["instance_feat"], inputs["in_src"], INST_N)
    feat1 = prescale(inputs["node_feat"], inputs["ni_src"], NODE_N)
    feat2 = prescale(inputs["svc_feat"], inputs["sc_src"], SVC_N)

    rels = [
        # order matters: output rows are [node_out, inst_out, svc_out]
        _prep_relation(inputs["in_src"], inputs["in_dst"], INST_N, NODE_N,
                       feat0, rs_of(inputs["in_dst"], NODE_N), compact=True),
        _prep_relation(inputs["ni_src"], inputs["ni_dst"], NODE_N, INST_N,
                       feat1, rs_of(inputs["ni_dst"], INST_N), compact=False),
        _prep_relation(inputs["sc_src"], inputs["sc_dst"], SVC_N, SVC_N,
                       feat2, rs_of(inputs["sc_dst"], SVC_N), compact=False),
    ]
    Ws = [inputs["W_inst"], inputs["W_node"], inputs["W_svc"]]
    bs = [inputs["b_inst"], inputs["b_node"], inputs["b_svc"]]

    umax = _rup(max(c["n_units"] for c in rels[0]["cores"]) + 2, 16)
    nblk_tot = sum(r["nblk"] for r in rels)
    nidx_tot = nblk_tot * BLK

    W_cat = np.concatenate([np.asarray(w, np.float32) for w in Ws], axis=1)
    b_col = np.stack([np.asarray(b, np.float32) for b in bs], axis=1)  # [128,3]

    # ramp width: max tile span of any block
    kmax = max(int(r["span"].max()) for r in rels)
    assert kmax * 128 <= 2048, f"ramp {kmax * 128} not fp16-exact"
    iota_ramp = np.tile(np.arange(kmax * 128, dtype=np.float16), (128, 1))

    in_maps = []
    for c in range(NCORES):
        kidx = np.concatenate([r["cores"][c]["kidx"] for r in rels])
        assert kidx.max() < 32768
        idx16 = np.ascontiguousarray(kidx.astype(np.int16).reshape(-1, 16).T)
        idx_sb = np.tile(idx16, (8, 1))

        def blkmaj(name):
            v = np.concatenate([r["cores"][c][name] for r in rels])
            return np.ascontiguousarray(
                v.reshape(nblk_tot, BLK).T).astype(np.float32)

        def mk_tbl(tab, rows):
            out = np.zeros((rows, HID), np.float16)
            out[:len(tab)] = tab.astype(np.float16)
            return np.ascontiguousarray(out)

        in_maps.append({
            "tbl_in": mk_tbl(rels[0]["cores"][c]["table"], umax),
            "tbl_ni": mk_tbl(rels[1]["cores"][c]["table"], NODE_N + 2),
            "tbl_sc": mk_tbl(rels[2]["cores"][c]["table"], SVC_N + 2),
            "idx_sb": np.ascontiguousarray(idx_sb),
            "dA_sb": blkmaj("dA"),
            "dB_sb": blkmaj("dB"),
            "rA_sb": blkmaj("rA"),
            "rB_sb": blkmaj("rB"),
            "W_cat": np.ascontiguousarray(W_cat),
            "b_col": np.ascontiguousarray(b_col),
            "iota_ramp": np.ascontiguousarray(iota_ramp),
        })

    # per-relation gather chunk plan: small chunks at the ends (fast
    # pipeline fill / short compute tail), large in the middle (less fixed
    # SWDGE overhead).  Entries are (start_block, nblocks).
    plans = []
    for r in rels:
        nblk = r["nblk"]
        sizes = []
        rem = nblk
        ramp = [8, 16]
        for s in ramp:
            if rem <= s + 16:
                break
            sizes.append(s)
            rem -= s
        tail = [8, 8, 16]
        tail_take = []
        for s in tail:
            if rem <= s + 16:
                break
            tail_take.append(s)
            rem -= s
        while rem > 12:
            sizes.append(8)
            rem -= 8
        if rem > 0:
            sizes.append(rem)
        sizes += tail_take[::-1]
        assert sum(sizes) == nblk
        starts = np.concatenate([[0], np.cumsum(sizes)[:-1]]).astype(int)
        plans.append(list(zip(starts.tolist(), sizes)))
    cmax = max(s for p in plans for _, s in p)

    meta = dict(
        umax=umax, nblk_tot=nblk_tot, nidx_tot=nidx_tot, kmax=kmax,
        plans=plans, cmax=cmax,
        ntiles=[r["ntiles"] for r in rels],
        ngrps=[r["ngrp"] for r in rels],
        Ds=[r["D"] for r in rels],
        n_dsts=[r["n_dst"] for r in rels],
        nslots=[r["nslot"] for r in rels],
        nblks=[r["nblk"] for r in rels],
        bstarts=[r["bstart"].tolist() for r in rels],
        bends=[r["bend"].tolist() for r in rels],
        T0s=[r["T0"].tolist() for r in rels],
        spans=[r["span"].tolist() for r in rels],
        activeA=[r["activeA"] for r in rels],
        activeB=[r["activeB"] for r in rels],
        tbl_rows=[umax, NODE_N + 2, SVC_N + 2],
    )
    return meta, in_maps


def _build_program(meta):
    import concourse.bacc as bacc
    import concourse.mybir as mybir
    import concourse.tile as tile

    f16 = mybir.dt.float16
    f32 = mybir.dt.float32
    f32r = mybir.dt.float32r
    AF = mybir.ActivationFunctionType
    act_fn = AF.Lrelu if ACT_MODE == "lrelu" else AF.Relu

    nblk_tot, nidx_tot = meta["nblk_tot"], meta["nidx_tot"]
    kmax = meta["kmax"]
    cmax = meta["cmax"]
    GW = TP * 128  # epilogue group width in dst columns

    nc = bacc.Bacc("TRN2", target_bir_lowering=False, debug=False,
                   enable_asserts=False, num_devices=NCORES)

    tbl_d = [
        nc.dram_tensor(nm, [meta["tbl_rows"][i], HID], f16,
                       kind="ExternalInput")
        for i, nm in enumerate(["tbl_in", "tbl_ni", "tbl_sc"])
    ]
    idx_d = nc.dram_tensor("idx_sb", [128, nidx_tot // 16], mybir.dt.int16,
                           kind="ExternalInput")
    dA_d = nc.dram_tensor("dA_sb", [128, nblk_tot], f32, kind="ExternalInput")
    dB_d = nc.dram_tensor("dB_sb", [128, nblk_tot], f32, kind="ExternalInput")
    rA_d = nc.dram_tensor("rA_sb", [128, nblk_tot], f32, kind="ExternalInput")
    rB_d = nc.dram_tensor("rB_sb", [128, nblk_tot], f32, kind="ExternalInput")
    W_d = nc.dram_tensor("W_cat", [128, 3 * HID], f32, kind="ExternalInput")
    b_d = nc.dram_tensor("b_col", [128, 3], f32, kind="ExternalInput")
    ior_d = nc.dram_tensor("iota_ramp", [128, kmax * 128], f16,
                           kind="ExternalInput")

    out_d = [
        nc.dram_tensor(nm, [128, meta["ntiles"][i] * 128], f16,
                       kind="ExternalOutput")
        for i, nm in enumerate(["out_node", "out_inst", "out_svc"])
    ]

    with tile.TileContext(nc) as tc:
        with (
            tc.tile_pool(name="const", bufs=1) as const,
            tc.tile_pool(name="g", bufs=10) as gpool,
            tc.tile_pool(name="st", bufs=12) as stpool,
            tc.tile_pool(name="evac", bufs=4) as evac,
            tc.tile_pool(name="osb", bufs=4) as opool,
            tc.tile_pool(name="psA", bufs=6, space="PSUM") as psA,
            tc.tile_pool(name="psO", bufs=2, space="PSUM") as psO,
        ):
            # load the leading idx slice first so gathers start ASAP
            idx_t = const.tile([128, nidx_tot // 16], mybir.dt.int16)
            c0 = min(3 * 16 * BLK // 16, nidx_tot // 16)
            nc.sync.dma_start(idx_t[:, :c0], idx_d.ap()[:, :c0])
            dA_t = const.tile([128, nblk_tot], f32)
            nc.sync.dma_start(dA_t[:], dA_d.ap())
            dB_t = const.tile([128, nblk_tot], f32)
            nc.sync.dma_start(dB_t[:], dB_d.ap())
            rA_t = const.tile([128, nblk_tot], f32)
            nc.sync.dma_start(rA_t[:], rA_d.ap())
            rB_t = const.tile([128, nblk_tot], f32)
            nc.sync.dma_start(rB_t[:], rB_d.ap())
            ior_t = const.tile([128, kmax * 128], f16)
            nc.sync.dma_start(ior_t[:], ior_d.ap())
            W_t = const.tile([128, 3 * HID], f32)
            nc.sync.dma_start(W_t[:], W_d.ap())
            W_r = const.tile([128, 3 * HID], f32r)
            nc.scalar.copy(W_r[:], W_t[:])
            b_t = const.tile([128, 3], f32)
            nc.sync.dma_start(b_t[:], b_d.ap())
            if c0 < nidx_tot // 16:
                nc.sync.dma_start(idx_t[:, c0:], idx_d.ap()[:, c0:])

            g_tiles = {}    # (rel, local chunk) -> gather tile
            st_tiles = {}   # (block, lane, kg) -> one-hot [128, GW]

            def issue_gather(ci, rel, local_b0, cblk, rel_blk0):
                gt = gpool.tile([128, cmax, LANES * HID], f16, tag="g")
                nidx = cblk * BLK
                off16 = (rel_blk0 + local_b0) * BLK // 16
                in_ap = tbl_d[rel].ap()
                in_ap.ap[0] = [HID, meta["tbl_rows"][rel] - 1]
                in_ap.ap[1] = [1, LANES * HID]
                nc.gpsimd.dma_gather(
                    out_ap=gt[:, :cblk, :],
                    in_ap=in_ap,
                    idxs_ap=idx_t[:, off16:off16 + nidx // 16],
                    num_idxs=nidx,
                    num_idxs_reg=nidx,
                    elem_size=LANES * HID,
                    elem_step=HID,
                    single_packet=False,
                )
                g_tiles[ci] = gt

            def issue_st(gb, lane, wid, dl_t, rs_t, eng=None):
                # value-weighted one-hot: rs_dst * (dl == iota), one DVE op in
                # 4x_2p mode (fp16 packed in/out; f32 scalar APs are exempt).
                # Built once per (block, lane) covering the block's full tile
                # span; per-tile matmuls slice 128-column windows from it.
                st = stpool.tile([128, kmax * 128], f16, tag="st")
                (eng or nc.vector).tensor_scalar(
                    st[:, :wid], ior_t[:, :wid],
                    dl_t[:, gb:gb + 1], rs_t[:, gb:gb + 1],
                    mybir.AluOpType.is_equal, mybir.AluOpType.mult)
                st_tiles[(gb, lane)] = st

            # per-relation static state
            R = []
            blk_base = 0
            for rel in range(3):
                ngrp = meta["ngrps"][rel]
                nblk = meta["nblks"][rel]
                bstart = meta["bstarts"][rel]
                bend = meta["bends"][rel]
                T0 = meta["T0s"][rel]
                actA = meta["activeA"][rel]
                actB = meta["activeB"][rel]
                plan = meta["plans"][rel]
                chunk_of = {}
                for pi, (pb, ps) in enumerate(plan):
                    for b in range(pb, pb + ps):
                        chunk_of[b] = pi
                # minimal one-hot width per (block, lane): widest active k
                kneed = {}
                for t in range(ngrp * TP):
                    for b in range(int(bstart[t]), int(bend[t])):
                        k = t - int(T0[b])
                        if actA[t, b]:
                            kneed[(b, 0)] = max(kneed.get((b, 0), 1), k + 1)
                        if actB[t, b]:
                            kneed[(b, 1)] = max(kneed.get((b, 1), 1), k + 1)
                R.append(dict(ngrp=ngrp, nblk=nblk, bstart=bstart, bend=bend,
                              T0=T0, actA=actA, actB=actB, plan=plan,
                              chunk_of=chunk_of, kneed=kneed,
                              blk_base=blk_base, osb=None, osb_g0=0))
                blk_base += nblk

            sched = [(0, rel, g) for rel in (1, 2, 0)
                     for g in range(R[rel]["ngrp"])]
            for _, rel, g in sched:
                ngrp = R[rel]["ngrp"]
                bstart, bend = R[rel]["bstart"], R[rel]["bend"]
                T0 = R[rel]["T0"]
                actA, actB = R[rel]["actA"], R[rel]["actB"]
                plan, chunk_of = R[rel]["plan"], R[rel]["chunk_of"]
                kneed = R[rel]["kneed"]
                blk_base = R[rel]["blk_base"]
                if True:
                    agg = psA.tile([128, GW], f32, tag="agg")
                    for kt in range(TP):
                        t = g * TP + kt
                        ems = []
                        for b in range(int(bstart[t]), int(bend[t])):
                            if actA[t, b]:
                                ems.append((b, 0))
                            if actB[t, b]:
                                ems.append((b, 1))
                        for i, (b, lane) in enumerate(ems):
                            gb = blk_base + b
                            pi = chunk_of[b]
                            ci = (rel, pi)
                            if ci not in g_tiles:
                                issue_gather(ci, rel, plan[pi][0],
                                             plan[pi][1], blk_base)
                            if (gb, lane) not in st_tiles:
                                issue_st(gb, lane, kneed[(b, lane)] * 128,
                                         dA_t if lane == 0 else dB_t,
                                         rA_t if lane == 0 else rB_t)
                            k = t - int(T0[b])
                            cj = b - plan[pi][0]
                            nc.tensor.matmul(
                                agg[:, kt * 128:(kt + 1) * 128],
                                g_tiles[ci][:, cj, lane * HID:(lane + 1) * HID],
                                st_tiles[(gb, lane)][:, k * 128:(k + 1) * 128],
                                start=(i == 0), stop=(i == len(ems) - 1),
                                skip_group_check=True)
                    # epilogue: po[h, d] = W.T @ agg_pair; Lrelu(po + b[h]);
                    # evacuation alternates Act/DVE to spread engine load
                    aggsb = evac.tile([128, GW], f32r, tag="evac")
                    if rel == 1 or g % 2 == 0:
                        nc.scalar.copy(aggsb[:], agg[:])
                    else:
                        nc.vector.tensor_copy(aggsb[:], agg[:])
                    po = psO.tile([128, GW], f32, tag="po")
                    nc.tensor.matmul(
                        po[:],
                        W_r[:, rel * HID:(rel + 1) * HID],
                        aggsb[:],
                        start=True, stop=True)
                    og = g % (OUT_GRP // TP)
                    if og == 0:
                        osb_new = opool.tile([128, OUT_GRP * 128], f16,
                                             tag="osb")
                        R[rel]["osb"] = osb_new
                        R[rel]["osb_g0"] = g
                    osb = R[rel]["osb"]
                    nc.scalar.activation(
                        osb[:, og * GW:(og + 1) * GW], po[:], act_fn,
                        bias=b_t[:, rel:rel + 1], scale=1.0, alpha=0.01)
                    if og == OUT_GRP // TP - 1 or g == ngrp - 1:
                        cols = (g - R[rel]["osb_g0"] + 1) * GW
                        dst = out_d[rel].ap()[:, R[rel]["osb_g0"] * GW:
                                              R[rel]["osb_g0"] * GW + cols]
                        nc.sync.dma_start(dst, osb[:, :cols])

    nc.compile()
    return nc


def _run(nc, in_maps, trace=False, **kw):
    from concourse import bass_utils
    res = bass_utils.run_bass_kernel_spmd(
        nc, in_maps, core_ids=list(range(NCORES)), trace=trace, **kw)
    return res


def _assemble(results, meta):
    out = np.empty((NODE_N + INST_N + SVC_N, HID), np.float32)
    offs = [0, NODE_N, NODE_N + INST_N]
    names = ["out_node", "out_inst", "out_svc"]
    for rel in range(3):
        D, n_dst = meta["Ds"][rel], meta["n_dsts"][rel]
        ntiles = meta["ntiles"][rel]
        for c in range(NCORES):
            lo = c * D
            n = max(0, min(D, n_dst - lo))
            if n > 0:
                arr = results[c][names[rel]]  # [128 h, ntiles*128 d] fp16
                rows = np.ascontiguousarray(
                    arr.reshape(128, ntiles, 128).transpose(1, 2, 0)
                ).reshape(-1, HID)[:n].astype(np.float32)
                out[offs[rel] + lo: offs[rel] + lo + n] = rows
    return out


def kernel(**inputs):
    import hashlib
    key = "prog"
    h = hashlib.sha1()
    for k in ("sc_src", "sc_dst", "in_src", "in_dst", "ni_src", "ni_dst"):
        h.update(np.ascontiguousarray(np.asarray(inputs[k], np.int32)).tobytes())
    sig = h.hexdigest()
    meta, in_maps = _build_host_data(inputs)
    if key in _cache and _cache[key][0] == sig:
        _, nc, _ = _cache[key]
    else:
        nc = _build_program(meta)
        _cache[key] = (sig, nc, meta)
    res = _run(nc, in_maps)
    return _assemble(res.results, meta)


# revision 54
# speedup vs baseline: 1.0093x; 1.0093x over previous
"""Trainium2 Bass kernel for a heterogeneous GraphConv layer (3 relations).

out = concat([leaky(GC(inst_feat, W_inst, in_*)),     # -> node   (10000)
              leaky(GC(node_feat, W_node, ni_*)),     # -> inst   (100000)
              leaky(GC(svc_feat,  W_svc,  sc_*))])    # -> svc    (20000)

GC(f, W, src, dst) = rsqrt(deg_d) * segsum_dst((rsqrt(deg_s)*f)[src]) @ W + b
(aggregation commutes with the dense @W, so we gather *raw scaled features*
and apply W once per destination tile group).

Strategy: destination-sharded across 8 NeuronCores.  The per-core source
tables are PERMUTED so that rows co-used by the same dst tile sit adjacently;
each dma_gather descriptor then uses an overlapping 512B window (elem 256
fp16 elems, step 128) that fetches TWO consecutive rows — one descriptor
serves up to two edges (lanes A/B).  Descriptor cost on TRN2 is identical
for 256B and 512B payloads, so pairing halves gather DMA time.  Gathers are
issued in small (8-block) chunks from a per-relation plan so transfers,
SWDGE descriptor generation and downstream compute pipeline finely.

Edges (sorted by dst) are packed densely into 128-slot blocks with per-tile
slot quotas (max over cores) so the block->tile map is identical on every
core.  Aggregation runs per GROUP of TP=2 dst tiles (256 PSUM columns):
per (block, lane, group) one DVE tensor_scalar builds a value-weighted
one-hot S[slot, d] = rs_dst * (dl == iota+off) (4x_2p DVE mode; the rsqrt
deg_d scale rides the one-hot so the epilogue needs no rank-1 bias matmul),
and PE accumulates agg[f, d] += G_lane.T @ S in PSUM.  Per group: one
matmul po[h, d] = W.T @ agg, one ScalarE Lrelu(po + b[h]) (bias per
partition in the [h, d] orientation), fp16 output DMA in the transposed
[h, d] layout (the host de-transposes and converts).
"""

import os as _os
from collections import defaultdict

import numpy as np

SVC_N, INST_N, NODE_N, HID = 20000, 100000, 10000, 128
NCORES = 8
BLK = 128           # slots per block (= PE contraction dim)
LANES = 2           # table rows per gather window (512B / 256B fp16 rows)
TP = 2              # dst tiles per aggregation group (256 PSUM columns)
CHUNK = int(_os.environ.get("GNN_CHUNK", "24"))   # blocks per gather instr
OUT_GRP = int(_os.environ.get("GNN_OUT_GRP", "16"))  # dst tiles per out DMA
ACT_MODE = "lrelu"

_cache = {}


def _cdiv(a, b):
    return (a + b - 1) // b


def _rup(a, b):
    return _cdiv(a, b) * b


def _sequence_sources(es, tile):
    """Order this core's used sources so same-tileset sources are adjacent."""
    n = len(es)
    starts = np.flatnonzero(np.r_[True, es[1:] != es[:-1]])
    ends = np.r_[starts[1:], n]
    keys = [tuple(tile[a:b]) for a, b in zip(starts, ends)]
    order = sorted(range(len(starts)), key=lambda i: keys[i])
    return order, starts, ends


def _prep_relation(src, dst, n_src, n_dst, feat_s, rs_d, compact):
    """Host-side sharding/packing for one relation."""
    src = np.asarray(src, np.int64)
    dst = np.asarray(dst, np.int64)

    D = _rup(_cdiv(n_dst, NCORES), 128)  # dst rows per core (padded)
    ntiles = D // 128
    assert ntiles % TP == 0

    cores = []
    for c in range(NCORES):
        lo = c * D
        m = (dst >= lo) & (dst < lo + D)
        es, ed = src[m], dst[m] - lo
        tl = ed >> 7
        order = np.lexsort((tl, es))
        es, ed, tl = es[order], ed[order], tl[order]

        uorder, starts, ends = _sequence_sources(es, tl)
        srcs_u = es[starts]
        nsrc_u = len(srcs_u)

        pos_of_u = np.empty(nsrc_u, np.int64)
        pos_of_u[uorder] = np.arange(nsrc_u)

        if compact:
            table = feat_s[srcs_u[uorder]]
            n_units = nsrc_u
        else:
            used_mask = np.zeros(n_src, bool)
            used_mask[srcs_u] = True
            perm = np.concatenate([srcs_u[uorder],
                                   np.flatnonzero(~used_mask)])
            table = feat_s[perm]
            n_units = n_src

        # slots per tile via the path-greedy pairing over table positions
        slot_k = [[] for _ in range(ntiles)]
        slot_dA = [[] for _ in range(ntiles)]
        slot_dB = [[] for _ in range(ntiles)]
        per_tile = defaultdict(list)  # tile -> list of (pos, [dst_locals])
        for ui in range(nsrc_u):
            a, b = starts[ui], ends[ui]
            p = pos_of_u[ui]
            t0 = a
            while t0 < b:
                t1 = t0
                while t1 < b and tl[t1] == tl[t0]:
                    t1 += 1
                per_tile[tl[t0]].append((p, ed[t0:t1]))
                t0 = t1
        for t, lst in per_tile.items():
            lst.sort(key=lambda x: x[0])
            sk, sa, sb = slot_k[t], slot_dA[t], slot_dB[t]
            prev_pos = -10
            prev_ds = []
            for p, ds in lst:
                ds = list(ds)
                if p == prev_pos + 1 and prev_ds:
                    npair = min(len(prev_ds), len(ds))
                    for i in range(npair):
                        sk.append(prev_pos)
                        sa.append(prev_ds[i])
                        sb.append(ds[i])
                    for d in prev_ds[npair:]:
                        sk.append(prev_pos)
                        sa.append(d)
                        sb.append(-1)
                    ds = ds[npair:]
                else:
                    for d in prev_ds:
                        sk.append(prev_pos)
                        sa.append(d)
                        sb.append(-1)
                prev_pos, prev_ds = p, ds
            for d in prev_ds:
                sk.append(prev_pos)
                sa.append(d)
                sb.append(-1)
            # paired slots first so lane-B tails can be skipped
            osort = sorted(range(len(sk)), key=lambda i: sb[i] < 0)
            slot_k[t] = [sk[i] for i in osort]
            slot_dA[t] = [sa[i] for i in osort]
            slot_dB[t] = [sb[i] for i in osort]

        cores.append(dict(slot_k=slot_k, slot_dA=slot_dA, slot_dB=slot_dB,
                          table=table, n_units=n_units))

    # shared per-tile quotas and block map
    quota = np.zeros(ntiles, np.int64)
    for t in range(ntiles):
        quota[t] = max(max(len(cores[c]["slot_k"][t]) for c in range(NCORES)), 1)
    cum = np.concatenate([[0], np.cumsum(quota)])
    nslot = int(cum[-1])
    nslot_pad = _rup(nslot, BLK)
    nblk = nslot_pad // BLK
    bstart = (cum[:-1] // BLK).astype(np.int64)
    bend = np.minimum(-(-cum[1:] // BLK), nblk).astype(np.int64)
    bend = np.maximum(bend, bstart + 1)
    # T0(b): first tile covering block b; span(b): tiles covered
    T0 = np.zeros(nblk, np.int64)
    cur = 0
    for b in range(nblk):
        while bend[cur] <= b:
            cur += 1
        T0[b] = cur
    span = np.ones(nblk, np.int64)
    for t in range(ntiles):
        for b in range(int(bstart[t]), int(bend[t])):
            span[b] = max(span[b], t - T0[b] + 1)

    # per-core dst rsqrt-degree values (0 beyond n_dst)
    rs_core = []
    for c in range(NCORES):
        lo = c * D
        v = np.zeros(D, np.float32)
        n = max(0, min(D, n_dst - lo))
        if n > 0:
            v[:n] = rs_d[lo:lo + n]
        rs_core.append(v)

    ngrp = ntiles // TP
    activeA = np.zeros((ntiles, nblk), bool)
    activeB = np.zeros((ntiles, nblk), bool)
    for c in range(NCORES):
        d = cores[c]
        kidx = np.zeros(nslot_pad, np.int64)
        dA = np.full(nslot_pad, -1.0, np.float32)
        dB = np.full(nslot_pad, -1.0, np.float32)
        rA = np.zeros(nslot_pad, np.float32)
        rB = np.zeros(nslot_pad, np.float32)
        rsv = rs_core[c]
        for t in range(ntiles):
            off = int(cum[t])
            sk, sa, sb = d["slot_k"][t], d["slot_dA"][t], d["slot_dB"][t]
            for i in range(len(sk)):
                b = (off + i) // BLK
                shift = 128 * int(T0[b])
                kidx[off + i] = sk[i]
                dA[off + i] = sa[i] - shift
                rA[off + i] = rsv[sa[i]]
                activeA[t, b] = True
                if sb[i] >= 0:
                    dB[off + i] = sb[i] - shift
                    rB[off + i] = rsv[sb[i]]
                    activeB[t, b] = True
        # tail pads keep idx 0 (cost model charges num_idxs regardless; a
        # real gather keeps the SBUF block initialized -- NaN x 0 hazard)
        d["kidx"], d["dA"], d["dB"], d["rA"], d["rB"] = kidx, dA, dB, rA, rB
        del d["slot_k"], d["slot_dA"], d["slot_dB"]

    # force one active matmul per tile so every agg gets a start+stop
    for t in range(ntiles):
        if not activeA[t, bstart[t]:bend[t]].any() and \
           not activeB[t, bstart[t]:bend[t]].any():
            activeA[t, bstart[t]] = True

    return dict(cores=cores, ntiles=ntiles, ngrp=ngrp, D=D, n_dst=n_dst,
                nslot=nslot, nslot_pad=nslot_pad, nblk=nblk,
                bstart=bstart, bend=bend, T0=T0, span=span,
                activeA=activeA, activeB=activeB)


def _build_host_data(inputs):
    def prescale(feat, src, n_src):
        deg = np.maximum(np.bincount(np.asarray(src, np.int64),
                                     minlength=n_src), 1.0)
        return (np.asarray(feat, np.float32)
                / np.sqrt(deg)[:, None]).astype(np.float32)

    def rs_of(dstv, n_dst):
        deg = np.maximum(np.bincount(np.asarray(dstv, np.int64),
                                     minlength=n_dst), 1.0)
        return (1.0 / np.sqrt(deg)).astype(np.float32)

    feat0 = prescale(inputs["instance_feat"], inputs["in_src"], INST_N)
    feat1 = prescale(inputs["node_feat"], inputs["ni_src"], NODE_N)
    feat2 = prescale(inputs["svc_feat"], inputs["sc_src"], SVC_N)

    rels = [
        # order matters: output rows are [node_out, inst_out, svc_out]
        _prep_relation(inputs["in_src"], inputs["in_dst"], INST_N, NODE_N,
                       feat0, rs_of(inputs["in_dst"], NODE_N), compact=True),
        _prep_relation(inputs["ni_src"], inputs["ni_dst"], NODE_N, INST_N,
                       feat1, rs_of(inputs["ni_dst"], INST_N), compact=False),
        _prep_relation(inputs["sc_src"], inputs["sc_dst"], SVC_N, SVC_N,
                       feat2, rs_of(inputs["sc_dst"], SVC_N), compact=False),
    ]
    Ws = [inputs["W_inst"], inputs["W_node"], inputs["W_svc"]]
    bs = [inputs["b_inst"], inputs["b_node"], inputs["b_svc"]]

    umax = _rup(max(c["n_units"] for c in rels[0]["cores"]) + 2, 16)
    nblk_tot = sum(r["nblk"] for r in rels)
    nidx_tot = nblk_tot * BLK

    W_cat = np.concatenate([np.asarray(w, np.float32) for w in Ws], axis=1)
    b_col = np.stack([np.asarray(b, np.float32) for b in bs], axis=1)  # [128,3]

    # ramp width: max tile span of any block
    kmax = max(int(r["span"].max()) for r in rels)
    assert kmax * 128 <= 2048, f"ramp {kmax * 128} not fp16-exact"
    iota_ramp = np.tile(np.arange(kmax * 128, dtype=np.float16), (128, 1))

    in_maps = []
    for c in range(NCORES):
        kidx = np.concatenate([r["cores"][c]["kidx"] for r in rels])
        assert kidx.max() < 32768
        idx16 = np.ascontiguousarray(kidx.astype(np.int16).reshape(-1, 16).T)
        idx_sb = np.tile(idx16, (8, 1))

        def blkmaj(name):
            v = np.concatenate([r["cores"][c][name] for r in rels])
            return np.ascontiguousarray(
                v.reshape(nblk_tot, BLK).T).astype(np.float32)

        def mk_tbl(tab, rows):
            out = np.zeros((rows, HID), np.float16)
            out[:len(tab)] = tab.astype(np.float16)
            return np.ascontiguousarray(out)

        in_maps.append({
            "tbl_in": mk_tbl(rels[0]["cores"][c]["table"], umax),
            "tbl_ni": mk_tbl(rels[1]["cores"][c]["table"], NODE_N + 2),
            "tbl_sc": mk_tbl(rels[2]["cores"][c]["table"], SVC_N + 2),
            "idx_sb": np.ascontiguousarray(idx_sb),
            "dA_sb": blkmaj("dA"),
            "dB_sb": blkmaj("dB"),
            "rA_sb": blkmaj("rA"),
            "rB_sb": blkmaj("rB"),
            "W_cat": np.ascontiguousarray(W_cat),
            "b_col": np.ascontiguousarray(b_col),
            "iota_ramp": np.ascontiguousarray(iota_ramp),
        })

    # per-relation gather chunk plan: small chunks at the ends (fast
    # pipeline fill / short compute tail), large in the middle (less fixed
    # SWDGE overhead).  Entries are (start_block, nblocks).
    plans = []
    for r in rels:
        nblk = r["nblk"]
        sizes = []
        rem = nblk
        ramp = [8, 16]
        for s in ramp:
            if rem <= s + 16:
                break
            sizes.append(s)
            rem -= s
        tail = [8, 8, 16]
        tail_take = []
        for s in tail:
            if rem <= s + 16:
                break
            tail_take.append(s)
            rem -= s
        while rem > 12:
            sizes.append(8)
            rem -= 8
        if rem > 0:
            sizes.append(rem)
        sizes += tail_take[::-1]
        assert sum(sizes) == nblk
        starts = np.concatenate([[0], np.cumsum(sizes)[:-1]]).astype(int)
        plans.append(list(zip(starts.tolist(), sizes)))
    cmax = max(s for p in plans for _, s in p)

    meta = dict(
        umax=umax, nblk_tot=nblk_tot, nidx_tot=nidx_tot, kmax=kmax,
        plans=plans, cmax=cmax,
        ntiles=[r["ntiles"] for r in rels],
        ngrps=[r["ngrp"] for r in rels],
        Ds=[r["D"] for r in rels],
        n_dsts=[r["n_dst"] for r in rels],
        nslots=[r["nslot"] for r in rels],
        nblks=[r["nblk"] for r in rels],
        bstarts=[r["bstart"].tolist() for r in rels],
        bends=[r["bend"].tolist() for r in rels],
        T0s=[r["T0"].tolist() for r in rels],
        spans=[r["span"].tolist() for r in rels],
        activeA=[r["activeA"] for r in rels],
        activeB=[r["activeB"] for r in rels],
        tbl_rows=[umax, NODE_N + 2, SVC_N + 2],
    )
    return meta, in_maps


def _build_program(meta):
    import concourse.bacc as bacc
    import concourse.mybir as mybir
    import concourse.tile as tile

    f16 = mybir.dt.float16
    f32 = mybir.dt.float32
    f32r = mybir.dt.float32r
    AF = mybir.ActivationFunctionType
    act_fn = AF.Lrelu if ACT_MODE == "lrelu" else AF.Relu

    nblk_tot, nidx_tot = meta["nblk_tot"], meta["nidx_tot"]
    kmax = meta["kmax"]
    cmax = meta["cmax"]
    GW = TP * 128  # epilogue group width in dst columns

    nc = bacc.Bacc("TRN2", target_bir_lowering=False, debug=False,
                   enable_asserts=False, num_devices=NCORES)

    tbl_d = [
        nc.dram_tensor(nm, [meta["tbl_rows"][i], HID], f16,
                       kind="ExternalInput")
        for i, nm in enumerate(["tbl_in", "tbl_ni", "tbl_sc"])
    ]
    idx_d = nc.dram_tensor("idx_sb", [128, nidx_tot // 16], mybir.dt.int16,
                           kind="ExternalInput")
    dA_d = nc.dram_tensor("dA_sb", [128, nblk_tot], f32, kind="ExternalInput")
    dB_d = nc.dram_tensor("dB_sb", [128, nblk_tot], f32, kind="ExternalInput")
    rA_d = nc.dram_tensor("rA_sb", [128, nblk_tot], f32, kind="ExternalInput")
    rB_d = nc.dram_tensor("rB_sb", [128, nblk_tot], f32, kind="ExternalInput")
    W_d = nc.dram_tensor("W_cat", [128, 3 * HID], f32, kind="ExternalInput")
    b_d = nc.dram_tensor("b_col", [128, 3], f32, kind="ExternalInput")
    ior_d = nc.dram_tensor("iota_ramp", [128, kmax * 128], f16,
                           kind="ExternalInput")

    out_d = [
        nc.dram_tensor(nm, [128, meta["ntiles"][i] * 128], f16,
                       kind="ExternalOutput")
        for i, nm in enumerate(["out_node", "out_inst", "out_svc"])
    ]

    with tile.TileContext(nc) as tc:
        with (
            tc.tile_pool(name="const", bufs=1) as const,
            tc.tile_pool(name="g", bufs=10) as gpool,
            tc.tile_pool(name="st", bufs=12) as stpool,
            tc.tile_pool(name="evac", bufs=4) as evac,
            tc.tile_pool(name="osb", bufs=4) as opool,
            tc.tile_pool(name="psA", bufs=6, space="PSUM") as psA,
            tc.tile_pool(name="psO", bufs=2, space="PSUM") as psO,
        ):
            # load the leading idx slice first so gathers start ASAP
            idx_t = const.tile([128, nidx_tot // 16], mybir.dt.int16)
            c0 = min(3 * 16 * BLK // 16, nidx_tot // 16)
            nc.sync.dma_start(idx_t[:, :c0], idx_d.ap()[:, :c0])
            dA_t = const.tile([128, nblk_tot], f32)
            nc.sync.dma_start(dA_t[:], dA_d.ap())
            dB_t = const.tile([128, nblk_tot], f32)
            nc.sync.dma_start(dB_t[:], dB_d.ap())
            rA_t = const.tile([128, nblk_tot], f32)
            nc.sync.dma_start(rA_t[:], rA_d.ap())
            rB_t = const.tile([128, nblk_tot], f32)
            nc.sync.dma_start(rB_t[:], rB_d.ap())
            ior_t = const.tile([128, kmax * 128], f16)
            nc.sync.dma_start(ior_t[:], ior_d.ap())
            W_t = const.tile([128, 3 * HID], f32)
            nc.sync.dma_start(W_t[:], W_d.ap())
            W_r = const.tile([128, 3 * HID], f32r)
            nc.scalar.copy(W_r[:], W_t[:])
            b_t = const.tile([128, 3], f32)
            nc.sync.dma_start(b_t[:], b_d.ap())
            if c0 < nidx_tot // 16:
                nc.sync.dma_start(idx_t[:, c0:], idx_d.ap()[:, c0:])

            g_tiles = {}    # (rel, local chunk) -> gather tile
            st_tiles = {}   # (block, lane, kg) -> one-hot [128, GW]

            def issue_gather(ci, rel, local_b0, cblk, rel_blk0):
                gt = gpool.tile([128, cmax, LANES * HID], f16, tag="g")
                nidx = cblk * BLK
                off16 = (rel_blk0 + local_b0) * BLK // 16
                in_ap = tbl_d[rel].ap()
                in_ap.ap[0] = [HID, meta["tbl_rows"][rel] - 1]
                in_ap.ap[1] = [1, LANES * HID]
                nc.gpsimd.dma_gather(
                    out_ap=gt[:, :cblk, :],
                    in_ap=in_ap,
                    idxs_ap=idx_t[:, off16:off16 + nidx // 16],
                    num_idxs=nidx,
                    num_idxs_reg=nidx,
                    elem_size=LANES * HID,
                    elem_step=HID,
                    single_packet=False,
                )
                g_tiles[ci] = gt

            def issue_st(gb, lane, wid, dl_t, rs_t, eng=None):
                # value-weighted one-hot: rs_dst * (dl == iota), one DVE op in
                # 4x_2p mode (fp16 packed in/out; f32 scalar APs are exempt).
                # Built once per (block, lane) covering the block's full tile
                # span; per-tile matmuls slice 128-column windows from it.
                st = stpool.tile([128, kmax * 128], f16, tag="st")
                (eng or nc.vector).tensor_scalar(
                    st[:, :wid], ior_t[:, :wid],
                    dl_t[:, gb:gb + 1], rs_t[:, gb:gb + 1],
                    mybir.AluOpType.is_equal, mybir.AluOpType.mult)
                st_tiles[(gb, lane)] = st

            # per-relation static state
            R = []
            blk_base = 0
            for rel in range(3):
                ngrp = meta["ngrps"][rel]
                nblk = meta["nblks"][rel]
                bstart = meta["bstarts"][rel]
                bend = meta["bends"][rel]
                T0 = meta["T0s"][rel]
                actA = meta["activeA"][rel]
                actB = meta["activeB"][rel]
                plan = meta["plans"][rel]
                chunk_of = {}
                for pi, (pb, ps) in enumerate(plan):
                    for b in range(pb, pb + ps):
                        chunk_of[b] = pi
                # minimal one-hot width per (block, lane): widest active k
                kneed = {}
                for t in range(ngrp * TP):
                    for b in range(int(bstart[t]), int(bend[t])):
                        k = t - int(T0[b])
                        if actA[t, b]:
                            kneed[(b, 0)] = max(kneed.get((b, 0), 1), k + 1)
                        if actB[t, b]:
                            kneed[(b, 1)] = max(kneed.get((b, 1), 1), k + 1)
                R.append(dict(ngrp=ngrp, nblk=nblk, bstart=bstart, bend=bend,
                              T0=T0, actA=actA, actB=actB, plan=plan,
                              chunk_of=chunk_of, kneed=kneed,
                              blk_base=blk_base, osb=None, osb_g0=0))
                blk_base += nblk

            sched = [(0, rel, g) for rel in range(3)
                     for g in range(R[rel]["ngrp"])]
            for _, rel, g in sched:
                ngrp = R[rel]["ngrp"]
                bstart, bend = R[rel]["bstart"], R[rel]["bend"]
                T0 = R[rel]["T0"]
                actA, actB = R[rel]["actA"], R[rel]["actB"]
                plan, chunk_of = R[rel]["plan"], R[rel]["chunk_of"]
                kneed = R[rel]["kneed"]
                blk_base = R[rel]["blk_base"]
                if True:
                    agg = psA.tile([128, GW], f32, tag="agg")
                    for kt in range(TP):
                        t = g * TP + kt
                        ems = []
                        for b in range(int(bstart[t]), int(bend[t])):
                            if actA[t, b]:
                                ems.append((b, 0))
                            if actB[t, b]:
                                ems.append((b, 1))
                        for i, (b, lane) in enumerate(ems):
                            gb = blk_base + b
                            pi = chunk_of[b]
                            ci = (rel, pi)
                            if ci not in g_tiles:
                                issue_gather(ci, rel, plan[pi][0],
                                             plan[pi][1], blk_base)
                            if (gb, lane) not in st_tiles:
                                issue_st(gb, lane, kneed[(b, lane)] * 128,
                                         dA_t if lane == 0 else dB_t,
                                         rA_t if lane == 0 else rB_t)
                            k = t - int(T0[b])
                            cj = b - plan[pi][0]
                            nc.tensor.matmul(
                                agg[:, kt * 128:(kt + 1) * 128],
                                g_tiles[ci][:, cj, lane * HID:(lane + 1) * HID],
                                st_tiles[(gb, lane)][:, k * 128:(k + 1) * 128],
                                start=(i == 0), stop=(i == len(ems) - 1),
                                skip_group_check=True)
                    # epilogue: po[h, d] = W.T @ agg_pair; Lrelu(po + b[h]);
                    # evacuation alternates Act/DVE to spread engine load
                    aggsb = evac.tile([128, GW], f32r, tag="evac")
                    if rel == 1 and g % 2 == 0:
                        # Act saturates in the inst stretch; DVE idles there
                        nc.vector.tensor_copy(aggsb[:], agg[:])
                    else:
                        nc.scalar.copy(aggsb[:], agg[:])
                    po = psO.tile([128, GW], f32, tag="po")
                    nc.tensor.matmul(
                        po[:],
                        W_r[:, rel * HID:(rel + 1) * HID],
                        aggsb[:],
                        start=True, stop=True)
                    og = g % (OUT_GRP // TP)
                    if og == 0:
                        osb_new = opool.tile([128, OUT_GRP * 128], f16,
                                             tag="osb")
                        R[rel]["osb"] = osb_new
                        R[rel]["osb_g0"] = g
                    osb = R[rel]["osb"]
                    nc.scalar.activation(
                        osb[:, og * GW:(og + 1) * GW], po[:], act_fn,
                        bias=b_t[:, rel:rel + 1], scale=1.0, alpha=0.01)
                    if og == OUT_GRP // TP - 1 or g == ngrp - 1:
                        cols = (g - R[rel]["osb_g0"] + 1) * GW
                        dst = out_d[rel].ap()[:, R[rel]["osb_g0"] * GW:
                                              R[rel]["osb_g0"] * GW + cols]
                        nc.sync.dma_start(dst, osb[:, :cols])

    nc.compile()
    return nc


def _run(nc, in_maps, trace=False, **kw):
    from concourse import bass_utils
    res = bass_utils.run_bass_kernel_spmd(
        nc, in_maps, core_ids=list(range(NCORES)), trace=trace, **kw)
    return res


def _assemble(results, meta):
    out = np.empty((NODE_N + INST_N + SVC_N, HID), np.float32)
    offs = [0, NODE_N, NODE_N + INST_N]
    names = ["out_node", "out_inst", "out_svc"]
    for rel in range(3):
        D, n_dst = meta["Ds"][rel], meta["n_dsts"][rel]
        ntiles = meta["ntiles"][rel]
        for c in range(NCORES):
            lo = c * D
            n = max(0, min(D, n_dst - lo))
            if n > 0:
                arr = results[c][names[rel]]  # [128 h, ntiles*128 d] fp16
                rows = np.ascontiguousarray(
                    arr.reshape(128, ntiles, 128).transpose(1, 2, 0)
                ).reshape(-1, HID)[:n].astype(np.float32)
                out[offs[rel] + lo: offs[rel] + lo + n] = rows
    return out


def kernel(**inputs):
    import hashlib
    key = "prog"
    h = hashlib.sha1()
    for k in ("sc_src", "sc_dst", "in_src", "in_dst", "ni_src", "ni_dst"):
        h.update(np.ascontiguousarray(np.asarray(inputs[k], np.int32)).tobytes())
    sig = h.hexdigest()
    meta, in_maps = _build_host_data(inputs)
    if key in _cache and _cache[key][0] == sig:
        _, nc, _ = _cache[key]
    else:
        nc = _build_program(meta)
        _cache[key] = (sig, nc, meta)
    res = _run(nc, in_maps)
    return _assemble(res.results, meta)


# revision 55
# speedup vs baseline: 1.0513x; 1.0416x over previous
"""Trainium2 Bass kernel for a heterogeneous GraphConv layer (3 relations).

out = concat([leaky(GC(inst_feat, W_inst, in_*)),     # -> node   (10000)
              leaky(GC(node_feat, W_node, ni_*)),     # -> inst   (100000)
              leaky(GC(svc_feat,  W_svc,  sc_*))])    # -> svc    (20000)

GC(f, W, src, dst) = rsqrt(deg_d) * segsum_dst((rsqrt(deg_s)*f)[src]) @ W + b
(aggregation commutes with the dense @W, so we gather *raw scaled features*
and apply W once per destination tile group).

Strategy: destination-sharded across 8 NeuronCores.  The per-core source
tables are PERMUTED so that rows co-used by the same dst tile sit adjacently;
each dma_gather descriptor then uses an overlapping 512B window (elem 256
fp16 elems, step 128) that fetches TWO consecutive rows — one descriptor
serves up to two edges (lanes A/B).  Descriptor cost on TRN2 is identical
for 256B and 512B payloads, so pairing halves gather DMA time.  Gathers are
issued in small (8-block) chunks from a per-relation plan so transfers,
SWDGE descriptor generation and downstream compute pipeline finely.

Edges (sorted by dst) are packed densely into 128-slot blocks with per-tile
slot quotas (max over cores) so the block->tile map is identical on every
core.  Aggregation runs per GROUP of TP=2 dst tiles (256 PSUM columns):
per (block, lane, group) one DVE tensor_scalar builds a value-weighted
one-hot S[slot, d] = rs_dst * (dl == iota+off) (4x_2p DVE mode; the rsqrt
deg_d scale rides the one-hot so the epilogue needs no rank-1 bias matmul),
and PE accumulates agg[f, d] += G_lane.T @ S in PSUM.  Per group: one
matmul po[h, d] = W.T @ agg, one ScalarE Lrelu(po + b[h]) (bias per
partition in the [h, d] orientation), fp16 output DMA in the transposed
[h, d] layout (the host de-transposes and converts).
"""

import os as _os
from collections import defaultdict

import numpy as np

SVC_N, INST_N, NODE_N, HID = 20000, 100000, 10000, 128
NCORES = 8
BLK = 128           # slots per block (= PE contraction dim)
LANES = 2           # table rows per gather window (512B / 256B fp16 rows)
TP = 2              # dst tiles per aggregation group (256 PSUM columns)
CHUNK = int(_os.environ.get("GNN_CHUNK", "24"))   # blocks per gather instr
OUT_GRP = int(_os.environ.get("GNN_OUT_GRP", "16"))  # dst tiles per out DMA
ACT_MODE = "lrelu"

_cache = {}


def _cdiv(a, b):
    return (a + b - 1) // b


def _rup(a, b):
    return _cdiv(a, b) * b


def _sequence_sources(es, tile):
    """Order this core's used sources so same-tileset sources are adjacent."""
    n = len(es)
    starts = np.flatnonzero(np.r_[True, es[1:] != es[:-1]])
    ends = np.r_[starts[1:], n]
    keys = [tuple(tile[a:b]) for a, b in zip(starts, ends)]
    order = sorted(range(len(starts)), key=lambda i: keys[i])
    return order, starts, ends


def _prep_relation(src, dst, n_src, n_dst, feat_s, rs_d, compact):
    """Host-side sharding/packing for one relation."""
    src = np.asarray(src, np.int64)
    dst = np.asarray(dst, np.int64)

    D = _rup(_cdiv(n_dst, NCORES), 128)  # dst rows per core (padded)
    ntiles = D // 128
    assert ntiles % TP == 0

    cores = []
    for c in range(NCORES):
        lo = c * D
        m = (dst >= lo) & (dst < lo + D)
        es, ed = src[m], dst[m] - lo
        tl = ed >> 7
        order = np.lexsort((tl, es))
        es, ed, tl = es[order], ed[order], tl[order]

        uorder, starts, ends = _sequence_sources(es, tl)
        srcs_u = es[starts]
        nsrc_u = len(srcs_u)

        pos_of_u = np.empty(nsrc_u, np.int64)
        pos_of_u[uorder] = np.arange(nsrc_u)

        if compact:
            table = feat_s[srcs_u[uorder]]
            n_units = nsrc_u
        else:
            used_mask = np.zeros(n_src, bool)
            used_mask[srcs_u] = True
            perm = np.concatenate([srcs_u[uorder],
                                   np.flatnonzero(~used_mask)])
            table = feat_s[perm]
            n_units = n_src

        # slots per tile via the path-greedy pairing over table positions
        slot_k = [[] for _ in range(ntiles)]
        slot_dA = [[] for _ in range(ntiles)]
        slot_dB = [[] for _ in range(ntiles)]
        per_tile = defaultdict(list)  # tile -> list of (pos, [dst_locals])
        for ui in range(nsrc_u):
            a, b = starts[ui], ends[ui]
            p = pos_of_u[ui]
            t0 = a
            while t0 < b:
                t1 = t0
                while t1 < b and tl[t1] == tl[t0]:
                    t1 += 1
                per_tile[tl[t0]].append((p, ed[t0:t1]))
                t0 = t1
        for t, lst in per_tile.items():
            lst.sort(key=lambda x: x[0])
            sk, sa, sb = slot_k[t], slot_dA[t], slot_dB[t]
            prev_pos = -10
            prev_ds = []
            for p, ds in lst:
                ds = list(ds)
                if p == prev_pos + 1 and prev_ds:
                    npair = min(len(prev_ds), len(ds))
                    for i in range(npair):
                        sk.append(prev_pos)
                        sa.append(prev_ds[i])
                        sb.append(ds[i])
                    for d in prev_ds[npair:]:
                        sk.append(prev_pos)
                        sa.append(d)
                        sb.append(-1)
                    ds = ds[npair:]
                else:
                    for d in prev_ds:
                        sk.append(prev_pos)
                        sa.append(d)
                        sb.append(-1)
                prev_pos, prev_ds = p, ds
            for d in prev_ds:
                sk.append(prev_pos)
                sa.append(d)
                sb.append(-1)
            # paired slots first so lane-B tails can be skipped
            osort = sorted(range(len(sk)), key=lambda i: sb[i] < 0)
            slot_k[t] = [sk[i] for i in osort]
            slot_dA[t] = [sa[i] for i in osort]
            slot_dB[t] = [sb[i] for i in osort]

        cores.append(dict(slot_k=slot_k, slot_dA=slot_dA, slot_dB=slot_dB,
                          table=table, n_units=n_units))

    # shared per-tile quotas and block map
    quota = np.zeros(ntiles, np.int64)
    for t in range(ntiles):
        quota[t] = max(max(len(cores[c]["slot_k"][t]) for c in range(NCORES)), 1)
    cum = np.concatenate([[0], np.cumsum(quota)])
    nslot = int(cum[-1])
    nslot_pad = _rup(nslot, BLK)
    nblk = nslot_pad // BLK
    bstart = (cum[:-1] // BLK).astype(np.int64)
    bend = np.minimum(-(-cum[1:] // BLK), nblk).astype(np.int64)
    bend = np.maximum(bend, bstart + 1)
    # T0(b): first tile covering block b; span(b): tiles covered
    T0 = np.zeros(nblk, np.int64)
    cur = 0
    for b in range(nblk):
        while bend[cur] <= b:
            cur += 1
        T0[b] = cur
    span = np.ones(nblk, np.int64)
    for t in range(ntiles):
        for b in range(int(bstart[t]), int(bend[t])):
            span[b] = max(span[b], t - T0[b] + 1)

    # per-core dst rsqrt-degree values (0 beyond n_dst)
    rs_core = []
    for c in range(NCORES):
        lo = c * D
        v = np.zeros(D, np.float32)
        n = max(0, min(D, n_dst - lo))
        if n > 0:
            v[:n] = rs_d[lo:lo + n]
        rs_core.append(v)

    ngrp = ntiles // TP
    activeA = np.zeros((ntiles, nblk), bool)
    activeB = np.zeros((ntiles, nblk), bool)
    for c in range(NCORES):
        d = cores[c]
        kidx = np.zeros(nslot_pad, np.int64)
        dA = np.full(nslot_pad, -1.0, np.float32)
        dB = np.full(nslot_pad, -1.0, np.float32)
        rA = np.zeros(nslot_pad, np.float32)
        rB = np.zeros(nslot_pad, np.float32)
        rsv = rs_core[c]
        for t in range(ntiles):
            off = int(cum[t])
            sk, sa, sb = d["slot_k"][t], d["slot_dA"][t], d["slot_dB"][t]
            for i in range(len(sk)):
                b = (off + i) // BLK
                shift = 128 * int(T0[b])
                kidx[off + i] = sk[i]
                dA[off + i] = sa[i] - shift
                rA[off + i] = rsv[sa[i]]
                activeA[t, b] = True
                if sb[i] >= 0:
                    dB[off + i] = sb[i] - shift
                    rB[off + i] = rsv[sb[i]]
                    activeB[t, b] = True
        # tail pads keep idx 0 (cost model charges num_idxs regardless; a
        # real gather keeps the SBUF block initialized -- NaN x 0 hazard)
        d["kidx"], d["dA"], d["dB"], d["rA"], d["rB"] = kidx, dA, dB, rA, rB
        del d["slot_k"], d["slot_dA"], d["slot_dB"]

    # force one active matmul per tile so every agg gets a start+stop
    for t in range(ntiles):
        if not activeA[t, bstart[t]:bend[t]].any() and \
           not activeB[t, bstart[t]:bend[t]].any():
            activeA[t, bstart[t]] = True

    return dict(cores=cores, ntiles=ntiles, ngrp=ngrp, D=D, n_dst=n_dst,
                nslot=nslot, nslot_pad=nslot_pad, nblk=nblk,
                bstart=bstart, bend=bend, T0=T0, span=span,
                activeA=activeA, activeB=activeB)


def _build_host_data(inputs):
    def prescale(feat, src, n_src):
        deg = np.maximum(np.bincount(np.asarray(src, np.int64),
                                     minlength=n_src), 1.0)
        return (np.asarray(feat, np.float32)
                / np.sqrt(deg)[:, None]).astype(np.float32)

    def rs_of(dstv, n_dst):
        deg = np.maximum(np.bincount(np.asarray(dstv, np.int64),
                                     minlength=n_dst), 1.0)
        return (1.0 / np.sqrt(deg)).astype(np.float32)

    feat0 = prescale(inputs["instance_feat"], inputs["in_src"], INST_N)
    feat1 = prescale(inputs["node_feat"], inputs["ni_src"], NODE_N)
    feat2 = prescale(inputs["svc_feat"], inputs["sc_src"], SVC_N)

    rels = [
        # order matters: output rows are [node_out, inst_out, svc_out]
        _prep_relation(inputs["in_src"], inputs["in_dst"], INST_N, NODE_N,
                       feat0, rs_of(inputs["in_dst"], NODE_N), compact=True),
        _prep_relation(inputs["ni_src"], inputs["ni_dst"], NODE_N, INST_N,
                       feat1, rs_of(inputs["ni_dst"], INST_N), compact=False),
        _prep_relation(inputs["sc_src"], inputs["sc_dst"], SVC_N, SVC_N,
                       feat2, rs_of(inputs["sc_dst"], SVC_N), compact=False),
    ]
    Ws = [inputs["W_inst"], inputs["W_node"], inputs["W_svc"]]
    bs = [inputs["b_inst"], inputs["b_node"], inputs["b_svc"]]

    umax = _rup(max(c["n_units"] for c in rels[0]["cores"]) + 2, 16)
    nblk_tot = sum(r["nblk"] for r in rels)
    nidx_tot = nblk_tot * BLK

    W_cat = np.concatenate([np.asarray(w, np.float32) for w in Ws], axis=1)
    b_col = np.stack([np.asarray(b, np.float32) for b in bs], axis=1)  # [128,3]

    # ramp width: max tile span of any block
    kmax = max(int(r["span"].max()) for r in rels)
    assert kmax * 128 <= 2048, f"ramp {kmax * 128} not fp16-exact"
    iota_ramp = np.tile(np.arange(kmax * 128, dtype=np.float16), (128, 1))

    in_maps = []
    for c in range(NCORES):
        kidx = np.concatenate([r["cores"][c]["kidx"] for r in rels])
        assert kidx.max() < 32768
        idx16 = np.ascontiguousarray(kidx.astype(np.int16).reshape(-1, 16).T)
        idx_sb = np.tile(idx16, (8, 1))

        def blkmaj(name):
            v = np.concatenate([r["cores"][c][name] for r in rels])
            return np.ascontiguousarray(
                v.reshape(nblk_tot, BLK).T).astype(np.float32)

        def mk_tbl(tab, rows):
            out = np.zeros((rows, HID), np.float16)
            out[:len(tab)] = tab.astype(np.float16)
            return np.ascontiguousarray(out)

        in_maps.append({
            "tbl_in": mk_tbl(rels[0]["cores"][c]["table"], umax),
            "tbl_ni": mk_tbl(rels[1]["cores"][c]["table"], NODE_N + 2),
            "tbl_sc": mk_tbl(rels[2]["cores"][c]["table"], SVC_N + 2),
            "idx_sb": np.ascontiguousarray(idx_sb),
            "dA_sb": blkmaj("dA"),
            "dB_sb": blkmaj("dB"),
            "rA_sb": blkmaj("rA"),
            "rB_sb": blkmaj("rB"),
            "W_cat": np.ascontiguousarray(W_cat),
            "b_col": np.ascontiguousarray(b_col),
            "iota_ramp": np.ascontiguousarray(iota_ramp),
        })

    # per-relation gather chunk plan: small chunks at the ends (fast
    # pipeline fill / short compute tail), large in the middle (less fixed
    # SWDGE overhead).  Entries are (start_block, nblocks).
    plans = []
    for r in rels:
        nblk = r["nblk"]
        sizes = []
        rem = nblk
        ramp = [8, 16]
        for s in ramp:
            if rem <= s + 16:
                break
            sizes.append(s)
            rem -= s
        tail = [8, 8, 16]
        tail_take = []
        for s in tail:
            if rem <= s + 16:
                break
            tail_take.append(s)
            rem -= s
        while rem > 12:
            sizes.append(8)
            rem -= 8
        if rem > 0:
            sizes.append(rem)
        sizes += tail_take[::-1]
        assert sum(sizes) == nblk
        starts = np.concatenate([[0], np.cumsum(sizes)[:-1]]).astype(int)
        plans.append(list(zip(starts.tolist(), sizes)))
    cmax = max(s for p in plans for _, s in p)

    meta = dict(
        umax=umax, nblk_tot=nblk_tot, nidx_tot=nidx_tot, kmax=kmax,
        plans=plans, cmax=cmax,
        ntiles=[r["ntiles"] for r in rels],
        ngrps=[r["ngrp"] for r in rels],
        Ds=[r["D"] for r in rels],
        n_dsts=[r["n_dst"] for r in rels],
        nslots=[r["nslot"] for r in rels],
        nblks=[r["nblk"] for r in rels],
        bstarts=[r["bstart"].tolist() for r in rels],
        bends=[r["bend"].tolist() for r in rels],
        T0s=[r["T0"].tolist() for r in rels],
        spans=[r["span"].tolist() for r in rels],
        activeA=[r["activeA"] for r in rels],
        activeB=[r["activeB"] for r in rels],
        tbl_rows=[umax, NODE_N + 2, SVC_N + 2],
    )
    return meta, in_maps


def _build_program(meta):
    import concourse.bacc as bacc
    import concourse.mybir as mybir
    import concourse.tile as tile

    f16 = mybir.dt.float16
    f32 = mybir.dt.float32
    f32r = mybir.dt.float32r
    AF = mybir.ActivationFunctionType
    act_fn = AF.Lrelu if ACT_MODE == "lrelu" else AF.Relu

    nblk_tot, nidx_tot = meta["nblk_tot"], meta["nidx_tot"]
    kmax = meta["kmax"]
    cmax = meta["cmax"]
    GW = TP * 128  # epilogue group width in dst columns

    nc = bacc.Bacc("TRN2", target_bir_lowering=False, debug=False,
                   enable_asserts=False, num_devices=NCORES)

    tbl_d = [
        nc.dram_tensor(nm, [meta["tbl_rows"][i], HID], f16,
                       kind="ExternalInput")
        for i, nm in enumerate(["tbl_in", "tbl_ni", "tbl_sc"])
    ]
    idx_d = nc.dram_tensor("idx_sb", [128, nidx_tot // 16], mybir.dt.int16,
                           kind="ExternalInput")
    dA_d = nc.dram_tensor("dA_sb", [128, nblk_tot], f32, kind="ExternalInput")
    dB_d = nc.dram_tensor("dB_sb", [128, nblk_tot], f32, kind="ExternalInput")
    rA_d = nc.dram_tensor("rA_sb", [128, nblk_tot], f32, kind="ExternalInput")
    rB_d = nc.dram_tensor("rB_sb", [128, nblk_tot], f32, kind="ExternalInput")
    W_d = nc.dram_tensor("W_cat", [128, 3 * HID], f32, kind="ExternalInput")
    b_d = nc.dram_tensor("b_col", [128, 3], f32, kind="ExternalInput")
    ior_d = nc.dram_tensor("iota_ramp", [128, kmax * 128], f16,
                           kind="ExternalInput")

    out_d = [
        nc.dram_tensor(nm, [128, meta["ntiles"][i] * 128], f16,
                       kind="ExternalOutput")
        for i, nm in enumerate(["out_node", "out_inst", "out_svc"])
    ]

    with tile.TileContext(nc) as tc:
        with (
            tc.tile_pool(name="const", bufs=1) as const,
            tc.tile_pool(name="g", bufs=10) as gpool,
            tc.tile_pool(name="st", bufs=12) as stpool,
            tc.tile_pool(name="evac", bufs=4) as evac,
            tc.tile_pool(name="osb", bufs=4) as opool,
            tc.tile_pool(name="psA", bufs=6, space="PSUM") as psA,
            tc.tile_pool(name="psO", bufs=2, space="PSUM") as psO,
        ):
            # load the leading idx slice first so gathers start ASAP
            idx_t = const.tile([128, nidx_tot // 16], mybir.dt.int16)
            c0 = min(3 * 16 * BLK // 16, nidx_tot // 16)
            nc.sync.dma_start(idx_t[:, :c0], idx_d.ap()[:, :c0])
            dA_t = const.tile([128, nblk_tot], f32)
            nc.sync.dma_start(dA_t[:], dA_d.ap())
            dB_t = const.tile([128, nblk_tot], f32)
            nc.sync.dma_start(dB_t[:], dB_d.ap())
            rA_t = const.tile([128, nblk_tot], f32)
            nc.sync.dma_start(rA_t[:], rA_d.ap())
            rB_t = const.tile([128, nblk_tot], f32)
            nc.sync.dma_start(rB_t[:], rB_d.ap())
            ior_t = const.tile([128, kmax * 128], f16)
            nc.sync.dma_start(ior_t[:], ior_d.ap())
            W_t = const.tile([128, 3 * HID], f32)
            nc.sync.dma_start(W_t[:], W_d.ap())
            W_r = const.tile([128, 3 * HID], f32r)
            nc.scalar.copy(W_r[:], W_t[:])
            b_t = const.tile([128, 3], f32)
            nc.sync.dma_start(b_t[:], b_d.ap())
            if c0 < nidx_tot // 16:
                nc.sync.dma_start(idx_t[:, c0:], idx_d.ap()[:, c0:])

            g_tiles = {}    # (rel, local chunk) -> gather tile
            st_tiles = {}   # (block, lane, kg) -> one-hot [128, GW]

            def issue_gather(ci, rel, local_b0, cblk, rel_blk0):
                gt = gpool.tile([128, cmax, LANES * HID], f16, tag="g")
                nidx = cblk * BLK
                off16 = (rel_blk0 + local_b0) * BLK // 16
                in_ap = tbl_d[rel].ap()
                in_ap.ap[0] = [HID, meta["tbl_rows"][rel] - 1]
                in_ap.ap[1] = [1, LANES * HID]
                nc.gpsimd.dma_gather(
                    out_ap=gt[:, :cblk, :],
                    in_ap=in_ap,
                    idxs_ap=idx_t[:, off16:off16 + nidx // 16],
                    num_idxs=nidx,
                    num_idxs_reg=nidx,
                    elem_size=LANES * HID,
                    elem_step=HID,
                    single_packet=False,
                )
                g_tiles[ci] = gt

            def issue_st(gb, lane, wid, dl_t, rs_t, eng=None):
                # value-weighted one-hot: rs_dst * (dl == iota), one DVE op in
                # 4x_2p mode (fp16 packed in/out; f32 scalar APs are exempt).
                # Built once per (block, lane) covering the block's full tile
                # span; per-tile matmuls slice 128-column windows from it.
                st = stpool.tile([128, kmax * 128], f16, tag="st")
                (eng or nc.vector).tensor_scalar(
                    st[:, :wid], ior_t[:, :wid],
                    dl_t[:, gb:gb + 1], rs_t[:, gb:gb + 1],
                    mybir.AluOpType.is_equal, mybir.AluOpType.mult)
                st_tiles[(gb, lane)] = st

            # per-relation static state
            R = []
            blk_base = 0
            for rel in range(3):
                ngrp = meta["ngrps"][rel]
                nblk = meta["nblks"][rel]
                bstart = meta["bstarts"][rel]
                bend = meta["bends"][rel]
                T0 = meta["T0s"][rel]
                actA = meta["activeA"][rel]
                actB = meta["activeB"][rel]
                plan = meta["plans"][rel]
                chunk_of = {}
                for pi, (pb, ps) in enumerate(plan):
                    for b in range(pb, pb + ps):
                        chunk_of[b] = pi
                # minimal one-hot width per (block, lane): widest active k
                kneed = {}
                for t in range(ngrp * TP):
                    for b in range(int(bstart[t]), int(bend[t])):
                        k = t - int(T0[b])
                        if actA[t, b]:
                            kneed[(b, 0)] = max(kneed.get((b, 0), 1), k + 1)
                        if actB[t, b]:
                            kneed[(b, 1)] = max(kneed.get((b, 1), 1), k + 1)
                R.append(dict(ngrp=ngrp, nblk=nblk, bstart=bstart, bend=bend,
                              T0=T0, actA=actA, actB=actB, plan=plan,
                              chunk_of=chunk_of, kneed=kneed,
                              blk_base=blk_base, osb=None, osb_g0=0))
                blk_base += nblk

            sched = [(0, rel, g) for rel in range(3)
                     for g in range(R[rel]["ngrp"])]
            for _, rel, g in sched:
                ngrp = R[rel]["ngrp"]
                bstart, bend = R[rel]["bstart"], R[rel]["bend"]
                T0 = R[rel]["T0"]
                actA, actB = R[rel]["actA"], R[rel]["actB"]
                plan, chunk_of = R[rel]["plan"], R[rel]["chunk_of"]
                kneed = R[rel]["kneed"]
                blk_base = R[rel]["blk_base"]
                if True:
                    agg = psA.tile([128, GW], f32, tag="agg")
                    for kt in range(TP):
                        t = g * TP + kt
                        ems = []
                        for b in range(int(bstart[t]), int(bend[t])):
                            if actA[t, b]:
                                ems.append((b, 0))
                            if actB[t, b]:
                                ems.append((b, 1))
                        for i, (b, lane) in enumerate(ems):
                            gb = blk_base + b
                            pi = chunk_of[b]
                            ci = (rel, pi)
                            if ci not in g_tiles:
                                issue_gather(ci, rel, plan[pi][0],
                                             plan[pi][1], blk_base)
                            if (gb, lane) not in st_tiles:
                                issue_st(gb, lane, kneed[(b, lane)] * 128,
                                         dA_t if lane == 0 else dB_t,
                                         rA_t if lane == 0 else rB_t)
                            k = t - int(T0[b])
                            cj = b - plan[pi][0]
                            nc.tensor.matmul(
                                agg[:, kt * 128:(kt + 1) * 128],
                                g_tiles[ci][:, cj, lane * HID:(lane + 1) * HID],
                                st_tiles[(gb, lane)][:, k * 128:(k + 1) * 128],
                                start=(i == 0), stop=(i == len(ems) - 1),
                                skip_group_check=True)
                    # epilogue: po[h, d] = W.T @ agg_pair; Lrelu(po + b[h]);
                    # evacuation alternates Act/DVE to spread engine load
                    aggsb = evac.tile([128, GW], f32r, tag="evac")
                    if rel == 1 or g % 2 == 0:
                        nc.scalar.copy(aggsb[:], agg[:])
                    else:
                        nc.vector.tensor_copy(aggsb[:], agg[:])
                    po = psO.tile([128, GW], f32, tag="po")
                    nc.tensor.matmul(
                        po[:],
                        W_r[:, rel * HID:(rel + 1) * HID],
                        aggsb[:],
                        start=True, stop=True)
                    og = g % (OUT_GRP // TP)
                    if og == 0:
                        osb_new = opool.tile([128, OUT_GRP * 128], f16,
                                             tag="osb")
                        R[rel]["osb"] = osb_new
                        R[rel]["osb_g0"] = g
                    osb = R[rel]["osb"]
                    nc.scalar.activation(
                        osb[:, og * GW:(og + 1) * GW], po[:], act_fn,
                        bias=b_t[:, rel:rel + 1], scale=1.0, alpha=0.01)
                    if og == OUT_GRP // TP - 1 or g == ngrp - 1:
                        cols = (g - R[rel]["osb_g0"] + 1) * GW
                        dst = out_d[rel].ap()[:, R[rel]["osb_g0"] * GW:
                                              R[rel]["osb_g0"] * GW + cols]
                        nc.sync.dma_start(dst, osb[:, :cols])

    nc.compile()
    return nc


def _run(nc, in_maps, trace=False, **kw):
    from concourse import bass_utils
    res = bass_utils.run_bass_kernel_spmd(
        nc, in_maps, core_ids=list(range(NCORES)), trace=trace, **kw)
    return res


def _assemble(results, meta):
    out = np.empty((NODE_N + INST_N + SVC_N, HID), np.float32)
    offs = [0, NODE_N, NODE_N + INST_N]
    names = ["out_node", "out_inst", "out_svc"]
    for rel in range(3):
        D, n_dst = meta["Ds"][rel], meta["n_dsts"][rel]
        ntiles = meta["ntiles"][rel]
        for c in range(NCORES):
            lo = c * D
            n = max(0, min(D, n_dst - lo))
            if n > 0:
                arr = results[c][names[rel]]  # [128 h, ntiles*128 d] fp16
                rows = np.ascontiguousarray(
                    arr.reshape(128, ntiles, 128).transpose(1, 2, 0)
                ).reshape(-1, HID)[:n].astype(np.float32)
                out[offs[rel] + lo: offs[rel] + lo + n] = rows
    return out


def kernel(**inputs):
    import hashlib
    key = "prog"
    h = hashlib.sha1()
    for k in ("sc_src", "sc_dst", "in_src", "in_dst", "ni_src", "ni_dst"):
        h.update(np.ascontiguousarray(np.asarray(inputs[k], np.int32)).tobytes())
    sig = h.hexdigest()
    meta, in_maps = _build_host_data(inputs)
    if key in _cache and _cache[key][0] == sig:
        _, nc, _ = _cache[key]
    else:
        nc = _build_program(meta)
        _cache[key] = (sig, nc, meta)
    res = _run(nc, in_maps)
    return _assemble(res.results, meta)
